# revision 1
# baseline (speedup 1.0000x reference)
"""ANFIS (M=512, F=2, R=M^2, B=256) distributed Bass kernel for 8 TRN2
NeuronCores.

Math restructuring: with mem0[b,i] = gauss(x[b,0]; mean0[i], sig0[i]) and
mem1[b,j] = gauss(x[b,1]; mean1[j], sig1[j]), the reference output is

  out[b] = (x0[b]*q0[b] + x1[b]*q1[b] + q2[b]) / (s0[b]*s1[b])

with q_W[b] = mem0[b,:] @ W @ mem1[b,:]^T for W in {cw0, cw1, cb} reshaped
to [M, M], s0 = sum_i mem0, s1 = sum_j mem1.  No [B, R] tensor is ever
materialized: per core this is two rank-2 outer-product matmuls for the
membership arguments, one bf16 [128]x[128,769] matmul against the packed
weight block [W0|W1|Wb|ones], and fused multiply-reduce epilogues.

Sharding: 8 cores = 4 i-chunks (128 rows) x 2 j-halves (256 cols) of the
[M, M] weight matrices.  Each core emits partial numerator / denominator
factors [256, 2]; the host sums the partials (all linear) and divides.

Raw bass (no Tile): manual semaphores, engines fully specialized:
  SYNC   : cols/x0-broadcast input DMAs, result DMAs
  ACT    : activation-table preheat, small/weight DMAs, Ln/Exp/Square chains
  PE     : rank-2 membership matmuls, U = mem0 @ [W0|W1|Wb|1] (bf16)
  DVE    : reciprocal, membership-arg prep, scalar_tensor_tensor epilogue
"""

import os
import numpy as np

import concourse.bass as bass
import concourse.mybir as mybir
from concourse.bass_utils import run_bass_kernel_spmd

import ml_dtypes

BF16_NP = ml_dtypes.bfloat16

M = 512
B = 256
N_CORES = 8
IC = 4
JHALF = 2
MI = M // IC  # 128
MJ = M // JHALF  # 256
NW = 3 * MJ + 1  # 769

F32 = mybir.dt.float32
BF16 = mybir.dt.bfloat16

_cache = {}


def build():
    nc = bass.Bass(target_bir_lowering=False, debug=False)

    mult = mybir.AluOpType.mult
    add = mybir.AluOpType.add
    sub = mybir.AluOpType.subtract
    EXP = mybir.ActivationFunctionType.Exp
    SQ = mybir.ActivationFunctionType.Square

    # cols  f32 [128, 6]:  mean0c | sigma0c | x0h0 | x0h1 | x1h0 | x1h1
    # wide  f32 [128, 256]: x0 broadcast
    # small f32 [1, 704]:   mean1h(256) | sigma1h(256) | lhsb bf16 bitcast(192)
    #                       (lhsb bf16 layout: x1(256) | -ones(128))
    # w     bf16 [128, 769]: W0 | W1 | Wb | ones
    cols_ext = nc.declare_dram_parameter("cols", [128, 6], F32, isOutput=False)
    wide_ext = nc.declare_dram_parameter("wide", [128, 256], F32, isOutput=False)
    small_ext = nc.declare_dram_parameter("small", [1, 704], F32, isOutput=False)
    w_ext = nc.declare_dram_parameter("w", [MI, NW], BF16, isOutput=False)
    out_ext = nc.declare_dram_parameter("out", [B, 2], F32, isOutput=True)

    from contextlib import ExitStack

    with ExitStack() as ctx:
        colst = ctx.enter_context(nc.sbuf_tensor("cols_s", [128, 6], F32))
        wide = ctx.enter_context(nc.sbuf_tensor("wide_s", [128, 256], F32))
        negm = ctx.enter_context(nc.sbuf_tensor("negm", [128, 1], F32))
        small = ctx.enter_context(nc.sbuf_tensor("small_s", [1, 704], F32))
        w = ctx.enter_context(nc.sbuf_tensor("w_s", [MI, NW], BF16))
        isig1 = ctx.enter_context(nc.sbuf_tensor("isig1", [1, MJ], F32))
        vb = ctx.enter_context(nc.sbuf_tensor("vb", [1, 2 * MJ], BF16))
        isig0 = ctx.enter_context(nc.sbuf_tensor("isig0", [128, 1], F32))
        sqa = ctx.enter_context(nc.sbuf_tensor("sqa", [128, B], F32))
        m0t = ctx.enter_context(nc.sbuf_tensor("m0t", [128, B], BF16))
        sqb0 = ctx.enter_context(nc.sbuf_tensor("sqb0", [128, MJ], F32))
        sqb1 = ctx.enter_context(nc.sbuf_tensor("sqb1", [128, MJ], F32))
        m1_0 = ctx.enter_context(nc.sbuf_tensor("m1_0", [128, MJ], F32))
        m1_1 = ctx.enter_context(nc.sbuf_tensor("m1_1", [128, MJ], F32))
        s1_0 = ctx.enter_context(nc.sbuf_tensor("s1_0", [128, 1], F32))
        s1_1 = ctx.enter_context(nc.sbuf_tensor("s1_1", [128, 1], F32))
        scr = ctx.enter_context(nc.sbuf_tensor("scr", [128, MJ], F32))
        scr2 = ctx.enter_context(nc.sbuf_tensor("scr2", [128, MJ], F32))
        scr3 = ctx.enter_context(nc.sbuf_tensor("scr3", [128, MJ], F32))
        q0x = ctx.enter_context(nc.sbuf_tensor("q0x", [128, 1], F32))
        q1x = ctx.enter_context(nc.sbuf_tensor("q1x", [128, 1], F32))
        q2 = ctx.enter_context(nc.sbuf_tensor("q2", [128, 1], F32))
        q0x1 = ctx.enter_context(nc.sbuf_tensor("q0x1", [128, 1], F32))
        q1x1 = ctx.enter_context(nc.sbuf_tensor("q1x1", [128, 1], F32))
        q21 = ctx.enter_context(nc.sbuf_tensor("q21", [128, 1], F32))
        res0 = ctx.enter_context(nc.sbuf_tensor("res0", [128, 2], F32))
        res1 = ctx.enter_context(nc.sbuf_tensor("res1", [128, 2], F32))
        pre = ctx.enter_context(nc.sbuf_tensor("pre", [1, 2], F32))
        lns = ctx.enter_context(nc.sbuf_tensor("lns", [1, MJ], F32))
        tb0 = ctx.enter_context(nc.psum_tensor("tb0", [128, MJ], F32))
        tb1 = ctx.enter_context(nc.psum_tensor("tb1", [128, MJ], F32))
        u0 = ctx.enter_context(nc.psum_tensor("u0", [128, NW], F32))
        u1 = ctx.enter_context(nc.psum_tensor("u1", [128, NW], F32))
        sd_small = ctx.enter_context(nc.semaphore("sd_small"))
        sd_small2 = ctx.enter_context(nc.semaphore("sd_small2"))
        sd_cols = ctx.enter_context(nc.semaphore("sd_cols"))
        sd_wide = ctx.enter_context(nc.semaphore("sd_wide"))
        sd_w = ctx.enter_context(nc.semaphore("sd_w"))
        sv = ctx.enter_context(nc.semaphore("sv"))
        sa = ctx.enter_context(nc.semaphore("sa"))
        sp = ctx.enter_context(nc.semaphore("sp"))
        so = ctx.enter_context(nc.semaphore("so"))
        sg = ctx.enter_context(nc.semaphore("sg"))
        block = ctx.enter_context(nc.Block())
        lhsb = small.ap()[0:1, 512:704].bitcast(BF16)  # [1, 384] bf16
        mean1 = small.ap()[0:1, 0:MJ]
        sigma1 = small.ap()[0:1, MJ:2 * MJ]
        x0b = wide.ap()[:, 0:B]
        mean0c = colst.ap()[:, 0:1]
        sigma0c = colst.ap()[:, 1:2]

        # Engines are deep-pipelined: every RAW hazard (cross- OR same-engine)
        # needs a semaphore edge.  Each engine increments its own counting
        # semaphore after data-producing instructions; consumers wait on the
        # producer's cumulative count.
        #
        # DVE (sv):  1 isig0 | 2 negm | 3 vb1
        #            4-6 stt q h0 | 7 den0 | 8-10 stt q h1 | 11 den1
        # ACT (sa):  1 prezero | 2 lns | 3 isig1 (vb row0) | 4 sqa | 5 m0t
        #            6 sqb0 | 7 m1_0 | 8 sqb1 | 9 m1_1
        # PE  (sp):  1 tb0 | 2 tb1 | 3 u0 | 4 u1
        # GPS (sg):  1 num0 | 2 num1

        LN = mybir.ActivationFunctionType.Ln

        @block.sync
        def _(sync):
            sync.dma_start(out=colst[:, :], in_=cols_ext[:, :]).then_inc(sd_cols, 16)
            sync.dma_start(out=wide[:, :], in_=wide_ext[:, :]).then_inc(sd_wide, 16)
            sync.wait_ge(sv, 7)
            sync.wait_ge(sg, 1)
            sync.dma_start(out=out_ext[0:128, :], in_=res0[:, :]).then_inc(so, 16)
            sync.wait_ge(sv, 12)
            sync.dma_start(out=out_ext[128:256, :], in_=res1[:, :]).then_inc(so, 16)

        @block.gpsimd
        def _(gpsimd):
            gpsimd.wait_ge(sv, 6)
            nc.gpsimd.tensor_scalar(res0.ap()[:, 0:1], q0x.ap(), q1x.ap(), q2.ap(),
                                    add, add).then_inc(sg, 1)


        @block.tensor
        def _(tensor):
            # membership rank-2 matmuls (bf16): t = x1*isig1 - mean1*isig1
            tensor.wait_ge(sv, 3)
            tensor.wait_ge(sd_small2, 16)
            nc.tensor.matmul(tb0.ap(), lhsb[0:1, 0:128], vb.ap()[0:1, 0:MJ],
                             start=True, stop=False)
            nc.tensor.matmul(tb0.ap(), lhsb[0:1, 256:384], vb.ap()[0:1, MJ:2 * MJ],
                             start=False, stop=True).then_inc(sp, 1)
            nc.tensor.matmul(tb1.ap(), lhsb[0:1, 128:256], vb.ap()[0:1, 0:MJ],
                             start=True, stop=False)
            nc.tensor.matmul(tb1.ap(), lhsb[0:1, 256:384], vb.ap()[0:1, MJ:2 * MJ],
                             start=False, stop=True).then_inc(sp, 1)
            tensor.wait_ge(sa, 5)
            tensor.wait_ge(sd_w, 16)
            nc.tensor.matmul(u0.ap()[:, 0:512], m0t.ap()[:, 0:128], w.ap()[:, 0:512],
                             start=True, stop=True)
            nc.tensor.matmul(u0.ap()[:, 512:NW], m0t.ap()[:, 0:128], w.ap()[:, 512:NW],
                             start=True, stop=True).then_inc(sp, 1)
            nc.tensor.matmul(u1.ap()[:, 0:512], m0t.ap()[:, 128:256], w.ap()[:, 0:512],
                             start=True, stop=True)
            nc.tensor.matmul(u1.ap()[:, 512:NW], m0t.ap()[:, 128:256], w.ap()[:, 512:NW],
                             start=True, stop=True).then_inc(sp, 1)

        @block.scalar
        def _(scalar):
            # memzero issues first: its PWP table load (one set covers
            # Exp/Ln/Square) runs while the DMAs below are in flight.
            nc.scalar.memzero(pre.ap()).then_inc(sa, 1)
            scalar.dma_start(out=small[:, 0:512],
                             in_=small_ext[:, 0:512]).then_inc(sd_small, 16)
            scalar.dma_start(out=small[:, 512:704],
                             in_=small_ext[:, 512:704]).then_inc(sd_small2, 16)
            scalar.dma_start(out=w[:, :], in_=w_ext[:, :]).then_inc(sd_w, 16)
            # isig1 row via exp(-ln(sigma1)), bf16 out straight into vb row0
            scalar.wait_ge(sd_small, 16)
            nc.scalar.activation(lns.ap(), sigma1, LN).then_inc(sa, 1)
            scalar.wait_ge(sa, 2)
            nc.scalar.activation(vb.ap()[0:1, 0:MJ], lns.ap(), EXP,
                                 scale=-1.0).then_inc(sa, 1)
            # mem0T: fused arg+square via per-partition scale/bias, then exp
            scalar.wait_ge(sv, 2)
            scalar.wait_ge(sd_wide, 16)
            nc.scalar.activation(sqa.ap(), x0b, SQ, bias=negm.ap(),
                                 scale=isig0.ap()).then_inc(sa, 1)
            scalar.wait_ge(sa, 4)
            nc.scalar.activation(m0t.ap(), sqa.ap(), EXP, scale=-1.0).then_inc(sa, 1)
            # mem1 halves: square + exp with running-sum accumulator
            scalar.wait_ge(sp, 1)
            nc.scalar.activation(sqb0.ap(), tb0.ap(), SQ).then_inc(sa, 1)
            scalar.wait_ge(sa, 6)
            nc.scalar.activation(m1_0.ap(), sqb0.ap(), EXP, scale=-1.0,
                                 accum_out=s1_0.ap()).then_inc(sa, 1)
            scalar.wait_ge(sp, 2)
            nc.scalar.activation(sqb1.ap(), tb1.ap(), SQ).then_inc(sa, 1)
            scalar.wait_ge(sa, 8)
            nc.scalar.activation(m1_1.ap(), sqb1.ap(), EXP, scale=-1.0,
                                 accum_out=s1_1.ap()).then_inc(sa, 1)

        @block.vector
        def _(vector):
            vector.wait_ge(sd_cols, 16)
            nc.vector.reciprocal(isig0.ap(), sigma0c).then_inc(sv, 1)
            vector.wait_ge(sv, 1)
            nc.vector.tensor_scalar(negm.ap(), mean0c, isig0.ap(), -1.0,
                                    mult, mult).then_inc(sv, 1)
            # vb row1 = mean1 * isig1 (bf16)
            vector.wait_ge(sa, 3)
            nc.vector.tensor_tensor(vb.ap()[0:1, MJ:2 * MJ], mean1,
                                    vb.ap()[0:1, 0:MJ], mult).then_inc(sv, 1)
            # epilogue half 0
            vector.wait_ge(sp, 3)
            vector.wait_ge(sa, 7)
            nc.vector.scalar_tensor_tensor(scr.ap(), u0.ap()[:, 0:MJ],
                                           colst.ap()[:, 2:3], m1_0.ap(),
                                           mult, mult, accum_out=q0x.ap()).then_inc(sv, 1)
            nc.vector.scalar_tensor_tensor(scr2.ap(), u0.ap()[:, MJ:2 * MJ],
                                           colst.ap()[:, 4:5], m1_0.ap(),
                                           mult, mult, accum_out=q1x.ap()).then_inc(sv, 1)
            nc.vector.scalar_tensor_tensor(scr3.ap(), u0.ap()[:, 2 * MJ:3 * MJ],
                                           1.0, m1_0.ap(),
                                           mult, mult, accum_out=q2.ap()).then_inc(sv, 1)
            nc.vector.tensor_tensor(res0.ap()[:, 1:2], u0.ap()[:, 768:769],
                                    s1_0.ap(), mult).then_inc(sv, 1)
            # epilogue half 1
            vector.wait_ge(sp, 4)
            vector.wait_ge(sa, 9)
            vector.wait_ge(sv, 7)
            nc.vector.scalar_tensor_tensor(scr.ap(), u1.ap()[:, 0:MJ],
                                           colst.ap()[:, 3:4], m1_1.ap(),
                                           mult, mult, accum_out=q0x1.ap()).then_inc(sv, 1)
            nc.vector.scalar_tensor_tensor(scr2.ap(), u1.ap()[:, MJ:2 * MJ],
                                           colst.ap()[:, 5:6], m1_1.ap(),
                                           mult, mult, accum_out=q1x1.ap()).then_inc(sv, 1)
            nc.vector.scalar_tensor_tensor(scr3.ap(), u1.ap()[:, 2 * MJ:3 * MJ],
                                           1.0, m1_1.ap(),
                                           mult, mult, accum_out=q21.ap()).then_inc(sv, 1)
            nc.vector.tensor_tensor(res1.ap()[:, 1:2], u1.ap()[:, 768:769],
                                    s1_1.ap(), mult).then_inc(sv, 1)
            vector.wait_ge(sv, 10)
            nc.vector.tensor_scalar(res1.ap()[:, 0:1], q0x1.ap(), q1x1.ap(),
                                    q21.ap(), add, add).then_inc(sv, 1)

    return nc


def shard_inputs(x, mean, sigma, cw, cb):
    x = np.ascontiguousarray(x, np.float32)
    mean = np.ascontiguousarray(mean, np.float32)
    sigma = np.ascontiguousarray(sigma, np.float32)
    cwr = np.ascontiguousarray(cw, np.float32).reshape(M, M, 2)
    cbr = np.ascontiguousarray(cb, np.float32).reshape(M, M)
    lhsb = np.concatenate([x[:, 1], -np.ones(128, np.float32)]).astype(BF16_NP)
    ones_col = np.ones((MI, 1), np.float32)
    x0b = np.broadcast_to(x[:, 0][None, :], (128, B))
    in_maps = []
    for c in range(N_CORES):
        ic, jh = c % IC, c // IC
        rs = slice(ic * MI, (ic + 1) * MI)
        cs = slice(jh * MJ, (jh + 1) * MJ)
        wv = np.concatenate(
            [cwr[rs, cs, 0], cwr[rs, cs, 1], cbr[rs, cs], ones_col],
            axis=1, dtype=np.float32,
        ).astype(BF16_NP)
        colsv = np.stack([
            mean[0, rs], sigma[0, rs],
            x[0:128, 0], x[128:256, 0], x[0:128, 1], x[128:256, 1],
        ], axis=1)
        smallv = np.concatenate(
            [mean[1, cs], sigma[1, cs], lhsb.view(np.float32)])[None, :]
        in_maps.append({
            "cols": np.ascontiguousarray(colsv, dtype=np.float32),
            "wide": np.ascontiguousarray(x0b, dtype=np.float32),
            "small": np.ascontiguousarray(smallv, dtype=np.float32),
            "w": np.ascontiguousarray(wv),
        })
    return in_maps


def combine(results):
    outs = np.stack([r["out"] for r in results])  # [8, 256, 2]
    num = outs[:, :, 0].sum(axis=0)
    den = outs[:, :, 1].sum(axis=0)
    return (num / den).astype(np.float32)[:, None]


def _ensure_ntff_hook():
    """The agent image's antenv lacks axon_hooks; build it from the boot
    helpers so run_bass_kernel_spmd(trace=True) can capture NTFF profiles."""
    import sys
    import types

    try:
        from antenv.axon_hooks import get_axon_ntff_profile_hook  # noqa: F401
        return
    except ImportError:
        pass
    mod = types.ModuleType("antenv.axon_hooks")
    holder = {}
    mod.set_axon_ntff_profile_hook = lambda h: holder.__setitem__("h", h)
    mod.get_axon_ntff_profile_hook = lambda: holder.get("h")
    try:
        from trn_agent_boot.trn_boot import _ntff_profile_via_ctypes

        hook = _ntff_profile_via_ctypes("/opt/axon/libaxon_pjrt.so")
        if hook is not None:
            holder["h"] = hook
    except Exception:
        pass
    sys.modules["antenv.axon_hooks"] = mod
    import antenv

    antenv.axon_hooks = mod


def run(inputs, trace=False, trace_kwargs=None):
    if trace:
        _ensure_ntff_hook()
    if "nc" not in _cache:
        _cache["nc"] = build()
    nc = _cache["nc"]
    in_maps = shard_inputs(**inputs)
    res = run_bass_kernel_spmd(
        nc, in_maps, core_ids=list(range(N_CORES)),
        trace=trace, **(trace_kwargs or {}),
    )
    return combine(res.results), res


def kernel(x, mean, sigma, cw, cb):
    out, _ = run(
        {"x": x, "mean": mean, "sigma": sigma, "cw": cw, "cb": cb},
        trace=bool(os.environ.get("ANFIS_TRACE")),
    )
    return out



# revision 6
# speedup vs baseline: 1.1298x; 1.1298x over previous
"""ANFIS (M=512, F=2, R=M^2, B=256) distributed Bass kernel for 8 TRN2
NeuronCores.

Math restructuring: with mem0[b,i] = gauss(x[b,0]; mean0[i], sig0[i]) and
mem1[b,j] = gauss(x[b,1]; mean1[j], sig1[j]), the reference output is

  out[b] = num[b] / den[b],   num = mem0 @ (x0*W0 + x1*W1 + Wb) @ mem1^T,
  den = (sum_i mem0)(sum_j mem1)

Per core (4 i-chunks x 2 j-halves of the [M, M] weight blocks):
  - m0t[i, b]  = DErf(isig0[i]*x0[b] - mean0[i]*isig0[i])   (one ACT op;
    Derivative_Erf(t) = (2/sqrt(pi)) exp(-t^2); the constant cancels in
    num/den so no correction is needed anywhere)
  - m0x0 = m0t * x0,  m0x1 = m0t * x1  (DVE; x1 broadcast via rank-1 PE matmul)
  - arg1[b, j] = x1[b]*isig1[j] - mean1[j]*isig1[j]  (one contraction-2
    matmul per batch half: lhsT=[x1; 1], rhs=[isig1; -mean1*isig1])
  - m1 = DErf(arg1) with accum_out -> s1 row sums
  - C[b, 0:257] = PSUM accumulation of three matmuls
        m0t  @ [Wb | ones], m0x0 @ [W0 | 0], m0x1 @ [W1 | 0]
    so C[:, 0:256] = x0*U0 + x1*U1 + Ub and C[:, 256] = s0.
  - num = rowsum(C[:, 0:256] * m1) (single STT w/ accum), den = C[:,256]*s1.
Host sums the 8 cores' [128, 2] partials and divides (all linear).

Raw bass (no Tile), engines specialized:
  SYNC : tiny + x0-block input DMAs, result DMAs
  ACT  : w DMA, table preheat, 3 Derivative_Erf activations
  PE   : 2 rank-2 membership-arg matmuls, 6 C-accumulation matmuls
  DVE  : m0x products, multiply-reduce epilogues
"""

import os
import numpy as np

import concourse.bass as bass
import concourse.mybir as mybir
from concourse.bass_utils import run_bass_kernel_spmd

import ml_dtypes

BF16_NP = ml_dtypes.bfloat16

M = 512
B = 256
N_CORES = 8
IC = 4
JHALF = 2
MI = M // IC  # 128
MJ = M // JHALF  # 256
NW = 3 * (MJ + 1)  # 771

F32 = mybir.dt.float32
BF16 = mybir.dt.bfloat16

_cache = {}


def build():
    nc = bass.Bass(target_bir_lowering=False, debug=False)

    mult = mybir.AluOpType.mult
    DERF = mybir.ActivationFunctionType.Derivative_Erf

    # x0c  bf16 [128, 262]: x0 broadcast (256) | f32-bitcast isig0, negm | f32 zero
    # mt   bf16 [2, 768]:  row0 = -mean1*isig1 | ones | x1
    #                      row1 = isig1         | x1   | junk
    # w    bf16 [128, 771]: Wb | ones | W0 | 0 | W1 | 0
    x0c_ext = nc.declare_dram_parameter("x0c", [MI, 262], BF16, isOutput=False)
    mt_ext = nc.declare_dram_parameter("mt", [2, 768], BF16, isOutput=False)
    w_ext = nc.declare_dram_parameter("w", [MI, NW], BF16, isOutput=False)
    out_ext = nc.declare_dram_parameter("out", [B, 2], F32, isOutput=True)

    from contextlib import ExitStack

    with ExitStack() as ctx:
        x0c = ctx.enter_context(nc.sbuf_tensor("x0c_s", [MI, 262], BF16))
        mt = ctx.enter_context(nc.sbuf_tensor("mt_s", [2, 768], BF16))
        w = ctx.enter_context(nc.sbuf_tensor("w_s", [MI, NW], BF16))
        m0t = ctx.enter_context(nc.sbuf_tensor("m0t", [128, B], BF16))
        m0x0 = ctx.enter_context(nc.sbuf_tensor("m0x0", [128, B], BF16))
        m0x1 = ctx.enter_context(nc.sbuf_tensor("m0x1", [128, B], BF16))
        m1_0 = ctx.enter_context(nc.sbuf_tensor("m1_0", [128, MJ], F32))
        m1_1 = ctx.enter_context(nc.sbuf_tensor("m1_1", [128, MJ], F32))
        s1_0 = ctx.enter_context(nc.sbuf_tensor("s1_0", [128, 1], F32))
        s1_1 = ctx.enter_context(nc.sbuf_tensor("s1_1", [128, 1], F32))
        scr0 = ctx.enter_context(nc.sbuf_tensor("scr0", [128, MJ], BF16))
        scr1 = ctx.enter_context(nc.sbuf_tensor("scr1", [128, MJ], BF16))
        res0 = ctx.enter_context(nc.sbuf_tensor("res0", [128, 2], F32))
        res1 = ctx.enter_context(nc.sbuf_tensor("res1", [128, 2], F32))
        pre = ctx.enter_context(nc.sbuf_tensor("pre", [1, 4], F32))
        x1p = ctx.enter_context(nc.psum_tensor("x1p", [128, MJ], F32))
        tb0 = ctx.enter_context(nc.psum_tensor("tb0", [128, MJ], F32))
        tb1 = ctx.enter_context(nc.psum_tensor("tb1", [128, MJ], F32))
        c0 = ctx.enter_context(nc.psum_tensor("c0", [128, MJ + 1], F32))
        c1 = ctx.enter_context(nc.psum_tensor("c1", [128, MJ + 1], F32))
        sd_t = ctx.enter_context(nc.semaphore("sd_t"))
        sd_x = ctx.enter_context(nc.semaphore("sd_x"))
        sd_w = ctx.enter_context(nc.semaphore("sd_w"))
        sv = ctx.enter_context(nc.semaphore("sv"))
        sa = ctx.enter_context(nc.semaphore("sa"))
        sp = ctx.enter_context(nc.semaphore("sp"))
        so = ctx.enter_context(nc.semaphore("so"))
        block = ctx.enter_context(nc.Block())

        x0bc = x0c.ap()[:, 0:256]
        consts = x0c.ap()[:, 256:260].bitcast(F32)  # [128, 2] f32
        isig0 = consts[:, 0:1]
        negm = consts[:, 1:2]
        zeroc = x0c.ap()[:, 260:262].bitcast(F32)  # [128, 1] f32 == 0
        vb2 = mt.ap()[0:2, 0:256]     # [-mean1*isig1; isig1]
        lhs2 = mt.ap()[0:2, 256:512]  # [ones; x1]
        onesr = mt.ap()[0:1, 256:384]  # [1, 128] ones
        x1row = mt.ap()[0:1, 512:768]  # [1, 256] x1

        # Engine-local counting semaphores; every cross-engine RAW edge
        # waits on the producer's cumulative count.
        # ACT (sa): 1 m0t | 2 m1_0(+s1_0) | 3 m1_1(+s1_1)
        # DVE (sv): 1 m0x0 | 2 m0x1 | 3 E0/num0 | 4 den0 | 5 E1/num1 | 6 den1
        # PE  (sp): 1 x1p | 2 tb0 | 3 tb1 | 4 c0 | 5 c1

        @block.sync
        def _(sync):
            sync.dma_start(out=mt[:, :], in_=mt_ext[:, :]).then_inc(sd_t, 16)
            sync.dma_start(out=x0c[:, :], in_=x0c_ext[:, :]).then_inc(sd_x, 16)
            sync.wait_ge(sv, 4)
            sync.dma_start(out=out_ext[0:128, :], in_=res0[:, :]).then_inc(so, 16)
            sync.wait_ge(sv, 6)
            sync.dma_start(out=out_ext[128:256, :], in_=res1[:, :]).then_inc(so, 16)

        @block.tensor
        def _(tensor):
            # x1 broadcast + membership-arg matmuls
            tensor.wait_ge(sd_t, 16)
            nc.tensor.matmul(x1p.ap(), onesr, x1row,
                             start=True, stop=True).then_inc(sp, 1)
            nc.tensor.matmul(tb0.ap(), lhs2[:, 0:128], vb2,
                             start=True, stop=True).then_inc(sp, 1)
            nc.tensor.matmul(tb1.ap(), lhs2[:, 128:256], vb2,
                             start=True, stop=True).then_inc(sp, 1)
            # C = m0t @ [Wb|1] + m0x0 @ [W0|0] + m0x1 @ [W1|0], per batch half
            tensor.wait_ge(sd_w, 16)
            tensor.wait_ge(sa, 1)
            nc.tensor.matmul(c0.ap(), m0t.ap()[:, 0:128], w.ap()[:, 0:257],
                             start=True, stop=False)
            tensor.wait_ge(sv, 1)
            nc.tensor.matmul(c0.ap(), m0x0.ap()[:, 0:128], w.ap()[:, 257:514],
                             start=False, stop=False)
            tensor.wait_ge(sv, 2)
            nc.tensor.matmul(c0.ap(), m0x1.ap()[:, 0:128], w.ap()[:, 514:771],
                             start=False, stop=True).then_inc(sp, 1)  # sp=4
            nc.tensor.matmul(c1.ap(), m0t.ap()[:, 128:256], w.ap()[:, 0:257],
                             start=True, stop=False)
            nc.tensor.matmul(c1.ap(), m0x0.ap()[:, 128:256], w.ap()[:, 257:514],
                             start=False, stop=False)
            nc.tensor.matmul(c1.ap(), m0x1.ap()[:, 128:256], w.ap()[:, 514:771],
                             start=False, stop=True).then_inc(sp, 1)

        @block.scalar
        def _(scalar):
            scalar.dma_start(out=w[:, :], in_=w_ext[:, :]).then_inc(sd_w, 16)
            # dummy op: forces the PWP table load before real work arrives
            nc.scalar.activation(pre.ap()[0:1, 2:4], pre.ap()[0:1, 0:2], DERF,
                                 bias=zeroc[0:1, 0:1])
            scalar.wait_ge(sd_x, 16)
            nc.scalar.activation(m0t.ap(), x0bc, DERF, bias=negm,
                                 scale=isig0).then_inc(sa, 1)
            scalar.wait_ge(sp, 2)
            nc.scalar.activation(m1_0.ap(), tb0.ap(), DERF, bias=zeroc,
                                 accum_out=s1_0.ap()).then_inc(sa, 1)
            scalar.wait_ge(sp, 3)
            nc.scalar.activation(m1_1.ap(), tb1.ap(), DERF, bias=zeroc,
                                 accum_out=s1_1.ap()).then_inc(sa, 1)

        @block.vector
        def _(vector):
            vector.wait_ge(sa, 1)
            nc.vector.tensor_tensor(m0x0.ap(), m0t.ap(), x0bc,
                                    mult).then_inc(sv, 1)
            vector.wait_ge(sp, 1)
            nc.vector.tensor_tensor(m0x1.ap(), m0t.ap(), x1p.ap(),
                                    mult).then_inc(sv, 1)
            vector.wait_ge(sp, 4)
            vector.wait_ge(sa, 2)
            nc.vector.scalar_tensor_tensor(scr0.ap(), c0.ap()[:, 0:256], 1.0,
                                           m1_0.ap(), mult, mult,
                                           accum_out=res0.ap()[:, 0:1]
                                           ).then_inc(sv, 1)
            nc.vector.tensor_tensor(res0.ap()[:, 1:2], c0.ap()[:, 256:257],
                                    s1_0.ap(), mult).then_inc(sv, 1)
            vector.wait_ge(sp, 5)
            vector.wait_ge(sa, 3)
            nc.vector.scalar_tensor_tensor(scr1.ap(), c1.ap()[:, 0:256], 1.0,
                                           m1_1.ap(), mult, mult,
                                           accum_out=res1.ap()[:, 0:1]
                                           ).then_inc(sv, 1)
            nc.vector.tensor_tensor(res1.ap()[:, 1:2], c1.ap()[:, 256:257],
                                    s1_1.ap(), mult).then_inc(sv, 1)

    return nc


def shard_inputs(x, mean, sigma, cw, cb):
    x = np.ascontiguousarray(x, np.float32)
    mean = np.ascontiguousarray(mean, np.float32)
    sigma = np.ascontiguousarray(sigma, np.float32)
    cwr = np.ascontiguousarray(cw, np.float32).reshape(M, M, 2)
    cbr = np.ascontiguousarray(cb, np.float32).reshape(M, M)
    isig = 1.0 / sigma
    nms = -mean * isig

    x0c_base = np.zeros((MI, 262), dtype=BF16_NP)
    x0c_base[:, 0:256] = np.broadcast_to(x[:, 0][None, :], (MI, B))

    mt_base = np.zeros((2, 768), dtype=BF16_NP)
    mt_base[0, 256:512] = 1.0
    mt_base[0, 512:768] = x[:, 1]
    mt_base[1, 256:512] = x[:, 1]

    ones_col = np.ones((MI, 1), np.float32)
    zero_col = np.zeros((MI, 1), np.float32)

    in_maps = []
    for c in range(N_CORES):
        ic, jh = c % IC, c // IC
        rs = slice(ic * MI, (ic + 1) * MI)
        cs = slice(jh * MJ, (jh + 1) * MJ)
        x0c_v = x0c_base.copy()
        consts = np.stack([isig[0, rs], nms[0, rs]], axis=1)  # [128, 2] f32
        x0c_v[:, 256:260] = consts.astype(np.float32).view(BF16_NP)
        mt_v = mt_base.copy()
        mt_v[0, 0:256] = nms[1, cs]
        mt_v[1, 0:256] = isig[1, cs]
        w_v = np.concatenate(
            [cbr[rs, cs], ones_col, cwr[rs, cs, 0], zero_col,
             cwr[rs, cs, 1], zero_col],
            axis=1, dtype=np.float32,
        ).astype(BF16_NP)
        in_maps.append({
            "x0c": np.ascontiguousarray(x0c_v),
            "mt": np.ascontiguousarray(mt_v),
            "w": np.ascontiguousarray(w_v),
        })
    return in_maps


def combine(results):
    outs = np.stack([r["out"] for r in results])  # [8, 256, 2]
    num = outs[:, :, 0].sum(axis=0)
    den = outs[:, :, 1].sum(axis=0)
    return (num / den).astype(np.float32)[:, None]


def _ensure_ntff_hook():
    """The agent image's antenv lacks axon_hooks; build it from the boot
    helpers so run_bass_kernel_spmd(trace=True) can capture NTFF profiles."""
    import sys
    import types

    try:
        from antenv.axon_hooks import get_axon_ntff_profile_hook  # noqa: F401
        return
    except ImportError:
        pass
    mod = types.ModuleType("antenv.axon_hooks")
    holder = {}
    mod.set_axon_ntff_profile_hook = lambda h: holder.__setitem__("h", h)
    mod.get_axon_ntff_profile_hook = lambda: holder.get("h")
    try:
        from trn_agent_boot.trn_boot import _ntff_profile_via_ctypes

        hook = _ntff_profile_via_ctypes("/opt/axon/libaxon_pjrt.so")
        if hook is not None:
            holder["h"] = hook
    except Exception:
        pass
    sys.modules["antenv.axon_hooks"] = mod
    import antenv

    antenv.axon_hooks = mod


def run(inputs, trace=False, trace_kwargs=None):
    if trace:
        _ensure_ntff_hook()
    if "nc" not in _cache:
        _cache["nc"] = build()
    nc = _cache["nc"]
    in_maps = shard_inputs(**inputs)
    res = run_bass_kernel_spmd(
        nc, in_maps, core_ids=list(range(N_CORES)),
        trace=trace, **(trace_kwargs or {}),
    )
    return combine(res.results), res


def kernel(x, mean, sigma, cw, cb):
    out, _ = run(
        {"x": x, "mean": mean, "sigma": sigma, "cw": cw, "cb": cb},
        trace=bool(os.environ.get("ANFIS_TRACE")),
    )
    return out


# revision 8
# speedup vs baseline: 1.1955x; 1.0582x over previous
"""ANFIS (M=512, F=2, R=M^2, B=256) distributed Bass kernel for 8 TRN2
NeuronCores.

Math restructuring: with mem0[b,i] = gauss(x[b,0]; mean0[i], sig0[i]) and
mem1[b,j] = gauss(x[b,1]; mean1[j], sig1[j]), the reference output is

  out[b] = num[b] / den[b],   num = mem0 @ (x0*W0 + x1*W1 + Wb) @ mem1^T,
  den = (sum_i mem0)(sum_j mem1)

Per core (4 i-chunks x 2 j-halves of the [M, M] weight blocks):
  - m0t[i, b]  = DErf(isig0[i]*x0[b] - mean0[i]*isig0[i])  (two ACT halves;
    Derivative_Erf(t) = (2/sqrt(pi)) exp(-t^2); the constant cancels in
    num/den so no correction is needed anywhere)
  - m0x0 = m0t * x0,  m0x1 = m0t * x1  (DVE, per batch half; x1 broadcast
    via rank-1 PE matmul)
  - arg1[b, j] = x1[b]*isig1[j] - mean1[j]*isig1[j]  (one contraction-2
    matmul per batch half: lhsT=[ones; x1], rhs=[-mean1*isig1; isig1])
  - m1 = DErf(arg1) with accum_out -> s1 row sums
  - C[b, 0:257] = PSUM accumulation of three matmuls
        m0t  @ [Wb | ones], m0x0 @ [W0 | 0], m0x1 @ [W1 | 0]
    so C[:, 0:256] = x0*U0 + x1*U1 + Ub and C[:, 256] = s0.
  - num = rowsum(C[:, 0:256] * m1) (single STT w/ accum), den = C[:,256]*s1.
Host sums the 8 cores' [128, 4] partials (num0|den0|num1|den1) and divides.

Raw bass (no Tile), engines specialized:
  SYNC : mt + x0-half input DMAs, merged result DMA
  ACT  : w DMAs (2 block slices), table preheat, 4 Derivative_Erf ops
  PE   : x1 broadcast, 2 rank-2 arg matmuls, 6 C-accumulation matmuls
  DVE  : m0x products (4 halves), multiply-reduce epilogues
"""

import os
import numpy as np

import concourse.bass as bass
import concourse.mybir as mybir
from concourse.bass_utils import run_bass_kernel_spmd

import ml_dtypes

BF16_NP = ml_dtypes.bfloat16

M = 512
B = 256
N_CORES = 8
IC = 4
JHALF = 2
MI = M // IC  # 128
MJ = M // JHALF  # 256
NW = 3 * (MJ + 1)  # 771

F32 = mybir.dt.float32
BF16 = mybir.dt.bfloat16

_cache = {}


def build():
    nc = bass.Bass(target_bir_lowering=False, debug=False)

    mult = mybir.AluOpType.mult
    DERF = mybir.ActivationFunctionType.Derivative_Erf

    # x0a  bf16 [128, 132]: x0[0:128] broadcast | f32-bitcast isig0, negm
    # x0b  bf16 [128, 128]: x0[128:256] broadcast
    # mt   bf16 [2, 768]:  row0 = -mean1*isig1 | ones | x1
    #                      row1 = isig1         | x1   | junk
    # w    bf16 [128, 771]: Wb | ones | W0 | 0 | W1 | 0
    x0a_ext = nc.declare_dram_parameter("x0a", [MI, 132], BF16, isOutput=False)
    x0b_ext = nc.declare_dram_parameter("x0b", [MI, 128], BF16, isOutput=False)
    mt_ext = nc.declare_dram_parameter("mt", [2, 768], BF16, isOutput=False)
    w_ext = nc.declare_dram_parameter("w", [MI, NW], BF16, isOutput=False)
    out_ext = nc.declare_dram_parameter("out", [MI, 4], F32, isOutput=True)

    from contextlib import ExitStack

    with ExitStack() as ctx:
        x0a = ctx.enter_context(nc.sbuf_tensor("x0a_s", [MI, 132], BF16))
        x0b = ctx.enter_context(nc.sbuf_tensor("x0b_s", [MI, 128], BF16))
        mt = ctx.enter_context(nc.sbuf_tensor("mt_s", [2, 768], BF16))
        w = ctx.enter_context(nc.sbuf_tensor("w_s", [MI, NW], BF16))
        m0t = ctx.enter_context(nc.sbuf_tensor("m0t", [128, B], BF16))
        m0x0 = ctx.enter_context(nc.sbuf_tensor("m0x0", [128, B], BF16))
        m0x1 = ctx.enter_context(nc.sbuf_tensor("m0x1", [128, B], BF16))
        m1_0 = ctx.enter_context(nc.sbuf_tensor("m1_0", [128, MJ], F32))
        m1_1 = ctx.enter_context(nc.sbuf_tensor("m1_1", [128, MJ], F32))
        s1_0 = ctx.enter_context(nc.sbuf_tensor("s1_0", [128, 1], F32))
        s1_1 = ctx.enter_context(nc.sbuf_tensor("s1_1", [128, 1], F32))
        scr0 = ctx.enter_context(nc.sbuf_tensor("scr0", [128, MJ], BF16))
        scr1 = ctx.enter_context(nc.sbuf_tensor("scr1", [128, MJ], BF16))
        res = ctx.enter_context(nc.sbuf_tensor("res", [128, 4], F32))
        pre = ctx.enter_context(nc.sbuf_tensor("pre", [1, 4], F32))
        x1p = ctx.enter_context(nc.psum_tensor("x1p", [128, MJ], F32))
        tb0 = ctx.enter_context(nc.psum_tensor("tb0", [128, MJ], F32))
        tb1 = ctx.enter_context(nc.psum_tensor("tb1", [128, MJ], F32))
        c0 = ctx.enter_context(nc.psum_tensor("c0", [128, MJ + 1], F32))
        c1 = ctx.enter_context(nc.psum_tensor("c1", [128, MJ + 1], F32))
        sd_t = ctx.enter_context(nc.semaphore("sd_t"))
        sd_xa = ctx.enter_context(nc.semaphore("sd_xa"))
        sd_xb = ctx.enter_context(nc.semaphore("sd_xb"))
        sd_w = ctx.enter_context(nc.semaphore("sd_w"))
        sv = ctx.enter_context(nc.semaphore("sv"))
        sa = ctx.enter_context(nc.semaphore("sa"))
        sp = ctx.enter_context(nc.semaphore("sp"))
        so = ctx.enter_context(nc.semaphore("so"))
        block = ctx.enter_context(nc.Block())

        x0abc = x0a.ap()[:, 0:128]
        consts = x0a.ap()[:, 128:132].bitcast(F32)  # [128, 2] f32
        isig0 = consts[:, 0:1]
        negm = consts[:, 1:2]
        vb2 = mt.ap()[0:2, 0:256]     # [-mean1*isig1; isig1]
        lhs2 = mt.ap()[0:2, 256:512]  # [ones; x1]
        onesr = mt.ap()[0:1, 256:384]  # [1, 128] ones
        x1row = mt.ap()[0:1, 512:768]  # [1, 256] x1

        # Engine-local counting semaphores; every cross-engine RAW edge
        # waits on the producer's cumulative count.
        # ACT (sa): 1 m0t_h0 | 2 m0t_h1 | 3 m1_0(+s1_0) | 4 m1_1(+s1_1)
        # DVE (sv): 1 m0x0h0 | 2 m0x1h0 | 3 m0x0h1 | 4 m0x1h1
        #           5 E0/num0 | 6 den0 | 7 E1/num1 | 8 den1
        # PE  (sp): 1 x1p | 2 tb0 | 3 tb1 | 4 c0 | 5 c1

        @block.sync
        def _(sync):
            sync.dma_start(out=mt[:, :], in_=mt_ext[:, :]).then_inc(sd_t, 16)
            sync.dma_start(out=x0a[:, :], in_=x0a_ext[:, :]).then_inc(sd_xa, 16)
            sync.dma_start(out=x0b[:, :], in_=x0b_ext[:, :]).then_inc(sd_xb, 16)
            sync.wait_ge(sv, 8)
            sync.dma_start(out=out_ext[:, :], in_=res[:, :]).then_inc(so, 16)

        @block.tensor
        def _(tensor):
            # x1 broadcast + membership-arg matmuls
            tensor.wait_ge(sd_t, 16)
            nc.tensor.matmul(x1p.ap(), onesr, x1row,
                             start=True, stop=True).then_inc(sp, 1)
            nc.tensor.matmul(tb0.ap(), lhs2[:, 0:128], vb2,
                             start=True, stop=True).then_inc(sp, 1)
            nc.tensor.matmul(tb1.ap(), lhs2[:, 128:256], vb2,
                             start=True, stop=True).then_inc(sp, 1)
            # C = m0t @ [Wb|1] + m0x0 @ [W0|0] + m0x1 @ [W1|0], per batch half
            tensor.wait_ge(sd_w, 16)
            tensor.wait_ge(sa, 1)
            nc.tensor.matmul(c0.ap(), m0t.ap()[:, 0:128], w.ap()[:, 0:257],
                             start=True, stop=False)
            tensor.wait_ge(sd_w, 32)
            tensor.wait_ge(sv, 1)
            nc.tensor.matmul(c0.ap(), m0x0.ap()[:, 0:128], w.ap()[:, 257:514],
                             start=False, stop=False)
            tensor.wait_ge(sv, 2)
            nc.tensor.matmul(c0.ap(), m0x1.ap()[:, 0:128], w.ap()[:, 514:771],
                             start=False, stop=True).then_inc(sp, 1)
            tensor.wait_ge(sa, 2)
            nc.tensor.matmul(c1.ap(), m0t.ap()[:, 128:256], w.ap()[:, 0:257],
                             start=True, stop=False)
            tensor.wait_ge(sv, 3)
            nc.tensor.matmul(c1.ap(), m0x0.ap()[:, 128:256], w.ap()[:, 257:514],
                             start=False, stop=False)
            tensor.wait_ge(sv, 4)
            nc.tensor.matmul(c1.ap(), m0x1.ap()[:, 128:256], w.ap()[:, 514:771],
                             start=False, stop=True).then_inc(sp, 1)

        @block.scalar
        def _(scalar):
            scalar.dma_start(out=w[:, 0:257],
                             in_=w_ext[:, 0:257]).then_inc(sd_w, 16)
            scalar.dma_start(out=w[:, 257:771],
                             in_=w_ext[:, 257:771]).then_inc(sd_w, 16)
            # dummy op: forces the PWP table load before real work arrives
            nc.scalar.activation(pre.ap()[0:1, 2:4], pre.ap()[0:1, 0:2], DERF)
            scalar.wait_ge(sd_xa, 16)
            nc.scalar.activation(m0t.ap()[:, 0:128], x0abc, DERF, bias=negm,
                                 scale=isig0).then_inc(sa, 1)
            scalar.wait_ge(sd_xb, 16)
            nc.scalar.activation(m0t.ap()[:, 128:256], x0b.ap(), DERF,
                                 bias=negm, scale=isig0).then_inc(sa, 1)
            scalar.wait_ge(sp, 2)
            nc.scalar.activation(m1_0.ap(), tb0.ap(), DERF,
                                 accum_out=s1_0.ap()).then_inc(sa, 1)
            scalar.wait_ge(sp, 3)
            nc.scalar.activation(m1_1.ap(), tb1.ap(), DERF,
                                 accum_out=s1_1.ap()).then_inc(sa, 1)

        @block.vector
        def _(vector):
            vector.wait_ge(sa, 1)
            nc.vector.tensor_tensor(m0x0.ap()[:, 0:128], m0t.ap()[:, 0:128],
                                    x0abc, mult).then_inc(sv, 1)
            vector.wait_ge(sp, 1)
            nc.vector.tensor_tensor(m0x1.ap()[:, 0:128], m0t.ap()[:, 0:128],
                                    x1p.ap()[:, 0:128], mult).then_inc(sv, 1)
            vector.wait_ge(sa, 2)
            nc.vector.tensor_tensor(m0x0.ap()[:, 128:256],
                                    m0t.ap()[:, 128:256],
                                    x0b.ap(), mult).then_inc(sv, 1)
            nc.vector.tensor_tensor(m0x1.ap()[:, 128:256],
                                    m0t.ap()[:, 128:256],
                                    x1p.ap()[:, 128:256], mult).then_inc(sv, 1)
            vector.wait_ge(sp, 4)
            vector.wait_ge(sa, 3)
            nc.vector.scalar_tensor_tensor(scr0.ap(), c0.ap()[:, 0:256], 1.0,
                                           m1_0.ap(), mult, mult,
                                           accum_out=res.ap()[:, 0:1]
                                           ).then_inc(sv, 1)
            nc.vector.tensor_tensor(res.ap()[:, 1:2], c0.ap()[:, 256:257],
                                    s1_0.ap(), mult).then_inc(sv, 1)
            vector.wait_ge(sp, 5)
            vector.wait_ge(sa, 4)
            nc.vector.scalar_tensor_tensor(scr1.ap(), c1.ap()[:, 0:256], 1.0,
                                           m1_1.ap(), mult, mult,
                                           accum_out=res.ap()[:, 2:3]
                                           ).then_inc(sv, 1)
            nc.vector.tensor_tensor(res.ap()[:, 3:4], c1.ap()[:, 256:257],
                                    s1_1.ap(), mult).then_inc(sv, 1)

    return nc


def shard_inputs(x, mean, sigma, cw, cb):
    x = np.ascontiguousarray(x, np.float32)
    mean = np.ascontiguousarray(mean, np.float32)
    sigma = np.ascontiguousarray(sigma, np.float32)
    cwr = np.ascontiguousarray(cw, np.float32).reshape(M, M, 2)
    cbr = np.ascontiguousarray(cb, np.float32).reshape(M, M)
    isig = 1.0 / sigma
    nms = -mean * isig

    x0a_base = np.zeros((MI, 132), dtype=BF16_NP)
    x0a_base[:, 0:128] = np.broadcast_to(x[0:128, 0][None, :], (MI, 128))
    x0b_v = np.ascontiguousarray(
        np.broadcast_to(x[128:256, 0][None, :], (MI, 128)).astype(BF16_NP))

    mt_base = np.zeros((2, 768), dtype=BF16_NP)
    mt_base[0, 256:512] = 1.0
    mt_base[0, 512:768] = x[:, 1]
    mt_base[1, 256:512] = x[:, 1]

    ones_col = np.ones((MI, 1), np.float32)
    zero_col = np.zeros((MI, 1), np.float32)

    in_maps = []
    for c in range(N_CORES):
        ic, jh = c % IC, c // IC
        rs = slice(ic * MI, (ic + 1) * MI)
        cs = slice(jh * MJ, (jh + 1) * MJ)
        x0a_v = x0a_base.copy()
        consts = np.stack([isig[0, rs], nms[0, rs]], axis=1)  # [128, 2] f32
        x0a_v[:, 128:132] = consts.astype(np.float32).view(BF16_NP)
        mt_v = mt_base.copy()
        mt_v[0, 0:256] = nms[1, cs]
        mt_v[1, 0:256] = isig[1, cs]
        w_v = np.concatenate(
            [cbr[rs, cs], ones_col, cwr[rs, cs, 0], zero_col,
             cwr[rs, cs, 1], zero_col],
            axis=1, dtype=np.float32,
        ).astype(BF16_NP)
        in_maps.append({
            "x0a": np.ascontiguousarray(x0a_v),
            "x0b": x0b_v,
            "mt": np.ascontiguousarray(mt_v),
            "w": np.ascontiguousarray(w_v),
        })
    return in_maps


def combine(results):
    outs = np.stack([r["out"] for r in results])  # [8, 128, 4]
    num = np.concatenate(
        [outs[:, :, 0].sum(axis=0), outs[:, :, 2].sum(axis=0)])
    den = np.concatenate(
        [outs[:, :, 1].sum(axis=0), outs[:, :, 3].sum(axis=0)])
    return (num / den).astype(np.float32)[:, None]


def _ensure_ntff_hook():
    """The agent image's antenv lacks axon_hooks; build it from the boot
    helpers so run_bass_kernel_spmd(trace=True) can capture NTFF profiles."""
    import sys
    import types

    try:
        from antenv.axon_hooks import get_axon_ntff_profile_hook  # noqa: F401
        return
    except ImportError:
        pass
    mod = types.ModuleType("antenv.axon_hooks")
    holder = {}
    mod.set_axon_ntff_profile_hook = lambda h: holder.__setitem__("h", h)
    mod.get_axon_ntff_profile_hook = lambda: holder.get("h")
    try:
        from trn_agent_boot.trn_boot import _ntff_profile_via_ctypes

        hook = _ntff_profile_via_ctypes("/opt/axon/libaxon_pjrt.so")
        if hook is not None:
            holder["h"] = hook
    except Exception:
        pass
    sys.modules["antenv.axon_hooks"] = mod
    import antenv

    antenv.axon_hooks = mod


def run(inputs, trace=False, trace_kwargs=None):
    if trace:
        _ensure_ntff_hook()
    if "nc" not in _cache:
        _cache["nc"] = build()
    nc = _cache["nc"]
    in_maps = shard_inputs(**inputs)
    res = run_bass_kernel_spmd(
        nc, in_maps, core_ids=list(range(N_CORES)),
        trace=trace, **(trace_kwargs or {}),
    )
    return combine(res.results), res


def kernel(x, mean, sigma, cw, cb):
    out, _ = run(
        {"x": x, "mean": mean, "sigma": sigma, "cw": cw, "cb": cb},
        trace=bool(os.environ.get("ANFIS_TRACE")),
    )
    return out
